# revision 2
# baseline (speedup 1.0000x reference)
"""Trainium2 Bass kernel for windowed (inverted-window) attention.

Problem: B=2, T=2048, C=2048, H=16 heads, D=128, WINDOW=512.
  q,k,v = x@Wq, x@Wk, x@Wv  (per-head reshape), RoPE on q,k,
  scores masked so positions INSIDE the causal window are masked out
  (attend only to j>i or j<i-511), softmax, o@Wo.

Sharding: 8 cores = 2 (batch) x 4 (head groups of 4 heads).
Each core computes its batch's 4 heads end-to-end plus a partial
output projection (row-chunk of Wo); host sums the 4 partials per batch.
"""

import sys
import numpy as np

for _p in ("/opt/trn_rl_repo",):
    if _p not in sys.path:
        sys.path.insert(0, _p)

import concourse.bass as bass  # noqa: E402
import concourse.mybir as mybir  # noqa: E402
from concourse.bacc import Bacc  # noqa: E402
from concourse.tile import TileContext  # noqa: E402
from concourse.bass import ts, ds  # noqa: E402
from concourse.bass_utils import run_bass_kernel_spmd  # noqa: E402

B, T, C, H, D = 2, 2048, 2048, 16, 128
HL = 4                # heads per core
NCORES = 8
WINDOW = 512
ROPE_BASE = 10000.0
TB = 512              # i/t block size (matmul free dim)
NTB = T // TB         # 4
CK = C // 128         # 16 contraction chunks for projections
NTC = T // 128        # 16 j-chunks / t-chunks
MASK_OFF = 1920       # master strip offset: off = i0 - j0 + MASK_OFF
MASK_W = 4352
F32 = mybir.dt.float32
F32R = mybir.dt.float32r
AF = mybir.ActivationFunctionType

USE_F32R = False      # use full-rate float32r matmuls (vs 1/4-rate float32)

_NC = None
TRACE = False
LAST_RESULT = None    # BassKernelResults of the most recent run (for test.py)


def _r(ap):
    """View an fp32 AP as float32r for full-rate PE matmuls."""
    return ap.bitcast(F32R) if USE_F32R else ap


def build_nc():
    nc = Bacc()
    xT = nc.declare_dram_parameter("xT", [C, T], F32, isOutput=False)
    wq = nc.declare_dram_parameter("wq", [C, HL * D], F32, isOutput=False)
    wk = nc.declare_dram_parameter("wk", [C, HL * D], F32, isOutput=False)
    wv = nc.declare_dram_parameter("wv", [C, HL * D], F32, isOutput=False)
    wo = nc.declare_dram_parameter("wo", [HL * D, C], F32, isOutput=False)
    cosx = nc.declare_dram_parameter("cosx", [128, T], F32, isOutput=False)
    sinx = nc.declare_dram_parameter("sinx", [128, T], F32, isOutput=False)
    maskm = nc.declare_dram_parameter("maskm", [128, MASK_W], F32, isOutput=False)
    out = nc.declare_dram_parameter("out", [T, C], F32, isOutput=True)

    xT_v = xT[:].rearrange("(co p) t -> p co t", p=128)   # [128, 16, T]
    wq_v = wq[:].rearrange("(co p) d -> p co d", p=128)   # [128, 16, 512]
    wk_v = wk[:].rearrange("(co p) d -> p co d", p=128)
    wv_v = wv[:].rearrange("(co p) d -> p co d", p=128)
    wo_v = wo[:].rearrange("(h p) c -> p h c", p=128)     # [128, 4, C]

    scale = float(1.0 / np.sqrt(D))

    with TileContext(nc) as tc:
        with tc.tile_pool(name="p0", bufs=1) as p0:
            oT = p0.tile([128, HL, T], F32)       # per-head o, transposed [d, t]

            with tc.tile_pool(name="p1", bufs=1) as p1:
                V = p1.tile([128, NTC, HL * D], F32)   # v natural [t, hd]
                QT = p1.tile([128, HL, T], F32)        # q transposed [d, t]
                KT = p1.tile([128, HL, T], F32)

                # ---- Phase A2: V = x @ Wv (natural layout) ----
                with (
                    tc.tile_pool(name="a2", bufs=1) as a2,
                    tc.tile_pool(name="a2x", bufs=2) as a2x,
                    tc.tile_pool(name="psV", bufs=2, space="PSUM") as psV,
                ):
                    wvt = a2.tile([128, CK, HL * D], F32)
                    nc.sync.dma_start(wvt[:], wv_v[:])
                    for tch in range(NTC):
                        xc = a2x.tile([128, CK, 128], F32, tag="xc")
                        nc.sync.dma_start(xc[:], xT_v[:, :, ts(tch, 128)])
                        ps = psV.tile([128, HL * D], F32, tag="psv")
                        for ck in range(CK):
                            nc.tensor.matmul(
                                ps[:], _r(xc[:, ck, :]), _r(wvt[:, ck, :]),
                                start=(ck == 0), stop=(ck == CK - 1),
                            )
                        nc.scalar.copy(V[:, tch, :], ps[:])

                # ---- Phase A1: QT/KT = (x @ Wq/Wk)^T with RoPE ----
                with (
                    tc.tile_pool(name="a1", bufs=1) as a1,
                    tc.tile_pool(name="a1w", bufs=2) as a1w,
                    tc.tile_pool(name="a1x", bufs=1) as a1x,
                    tc.tile_pool(name="a1t", bufs=2) as a1t,
                    tc.tile_pool(name="psA", bufs=4, space="PSUM") as psA,
                ):
                    cosb = a1.tile([128, T], F32)
                    nc.sync.dma_start(cosb[:], cosx[:])
                    sinb = a1.tile([128, T], F32)
                    nc.sync.dma_start(sinb[:], sinx[:])
                    for tb in range(NTB):
                        xtb = a1x.tile([128, CK, TB], F32, tag="xtb")
                        nc.sync.dma_start(xtb[:], xT_v[:, :, ts(tb, TB)])
                        for h in range(HL):
                            for w_v, OUTT in ((wq_v, QT), (wk_v, KT)):
                                wt = a1w.tile([128, CK, D], F32, tag="w")
                                nc.sync.dma_start(wt[:], w_v[:, :, ts(h, D)])
                                ps = psA.tile([128, TB], F32, tag="ps")
                                for ck in range(CK):
                                    nc.tensor.matmul(
                                        ps[:], _r(wt[:, ck, :]), _r(xtb[:, ck, :]),
                                        start=(ck == 0), stop=(ck == CK - 1),
                                    )
                                # RoPE: out = raw*cos + swap(raw)*sin_signed
                                raw = a1t.tile([128, TB], F32, tag="raw")
                                nc.scalar.copy(raw[:], ps[:])
                                sw = a1t.tile([128, TB], F32, tag="sw")
                                nc.sync.dma_start(sw[0:64, :], raw[64:128, :])
                                nc.sync.dma_start(sw[64:128, :], raw[0:64, :])
                                nc.vector.tensor_mul(sw[:], sw[:], sinb[:, ts(tb, TB)])
                                nc.vector.tensor_mul(raw[:], raw[:], cosb[:, ts(tb, TB)])
                                nc.vector.tensor_add(
                                    OUTT[:, h, ts(tb, TB)], sw[:], raw[:]
                                )

                # ---- Attention (per head, per i-block of 512 queries) ----
                with (
                    tc.tile_pool(name="att", bufs=1) as att,
                    tc.tile_pool(name="etp", bufs=18) as etp,
                    tc.tile_pool(name="smp", bufs=3) as smp,
                    tc.tile_pool(name="psS", bufs=3, space="PSUM") as psS,
                    tc.tile_pool(name="psO", bufs=2, space="PSUM") as psO,
                    tc.tile_pool(name="psZ", bufs=2, space="PSUM") as psZ,
                ):
                    maskb = att.tile([128, MASK_W], F32)
                    nc.sync.dma_start(maskb[:], maskm[:])
                    ones = att.tile([128, 128], F32)
                    nc.vector.memset(ones[:], 1.0)

                    for h in range(HL):
                        for ib in range(NTB):
                            ets = []
                            for c in range(NTC):
                                ps = psS.tile([128, TB], F32, tag="pss")
                                nc.tensor.matmul(
                                    ps[:],
                                    _r(KT[:, h, ts(c, 128)]),
                                    _r(QT[:, h, ts(ib, TB)]),
                                    start=True, stop=True,
                                )
                                et = etp.tile([128, TB], F32, tag="et")
                                nc.scalar.activation(et[:], ps[:], AF.Exp, scale=scale)
                                dd = ib * TB - c * 128
                                if -(WINDOW - 1) <= dd <= (WINDOW - 1) + 127:
                                    off = dd + MASK_OFF
                                    nc.vector.tensor_mul(
                                        et[:], et[:], maskb[:, ds(off, TB)]
                                    )
                                ets.append(et)
                            pso = psO.tile([128, TB], F32, tag="pso")
                            psz = psZ.tile([128, TB], F32, tag="psz")
                            for c in range(NTC):
                                nc.tensor.matmul(
                                    pso[:], _r(V[:, c, ts(h, D)]), _r(ets[c][:]),
                                    start=(c == 0), stop=(c == NTC - 1),
                                )
                                nc.tensor.matmul(
                                    psz[:], _r(ones[:]), _r(ets[c][:]),
                                    start=(c == 0), stop=(c == NTC - 1),
                                )
                            rz = smp.tile([128, TB], F32, tag="rz")
                            nc.vector.reciprocal(rz[:], psz[:])
                            nc.vector.tensor_mul(oT[:, h, ts(ib, TB)], pso[:], rz[:])

            # ---- Phase C: partial output projection ----
            with (
                tc.tile_pool(name="oc", bufs=1) as oc,
                tc.tile_pool(name="ocb", bufs=3) as ocb,
                tc.tile_pool(name="psC", bufs=4, space="PSUM") as psC,
            ):
                wot = oc.tile([128, HL, C], F32)
                nc.sync.dma_start(wot[:], wo_v[:])
                for tt in range(NTC):
                    for cb in range(NTB):
                        ps = psC.tile([128, TB], F32, tag="psc")
                        for h in range(HL):
                            nc.tensor.matmul(
                                ps[:],
                                _r(oT[:, h, ts(tt, 128)]),
                                _r(wot[:, h, ts(cb, TB)]),
                                start=(h == 0), stop=(h == HL - 1),
                            )
                        ob = ocb.tile([128, TB], F32, tag="ob")
                        nc.scalar.copy(ob[:], ps[:])
                        nc.sync.dma_start(out[ts(tt, 128), ts(cb, TB)], ob[:])

    nc.finalize()
    return nc


def _host_tables():
    inv_freq = (
        1.0 / (np.float32(ROPE_BASE) ** (np.arange(0, D, 2, dtype=np.float32) / np.float32(D)))
    ).astype(np.float32)
    t = np.arange(T, dtype=np.float32)
    freqs = (t[:, None] * inv_freq[None, :]).astype(np.float32)  # [T, 64]
    cos = np.cos(freqs).T.astype(np.float32)                     # [64, T]
    sin = np.sin(freqs).T.astype(np.float32)
    cosx = np.ascontiguousarray(np.concatenate([cos, cos], axis=0))      # [128, T]
    sinx = np.ascontiguousarray(np.concatenate([-sin, sin], axis=0))
    p = np.arange(128, dtype=np.int64)[:, None]
    u = np.arange(MASK_W, dtype=np.int64)[None, :]
    delta = u - MASK_OFF - p          # = i - j for tile offset
    allow = ~((delta >= 0) & (delta <= WINDOW - 1))
    maskm = np.ascontiguousarray(allow.astype(np.float32))
    return cosx, sinx, maskm


def kernel(x, Wq, Wk, Wv, Wo):
    global _NC, LAST_RESULT
    if _NC is None:
        _NC = build_nc()
    x = np.asarray(x, dtype=np.float32)
    Wq = np.asarray(Wq, dtype=np.float32)
    Wk = np.asarray(Wk, dtype=np.float32)
    Wv = np.asarray(Wv, dtype=np.float32)
    Wo = np.asarray(Wo, dtype=np.float32)
    cosx, sinx, maskm = _host_tables()
    in_maps = []
    for core in range(NCORES):
        b, hg = divmod(core, NCORES // B)
        sl = slice(hg * HL * D, (hg + 1) * HL * D)
        in_maps.append(
            {
                "xT": np.ascontiguousarray(x[b].T),
                "wq": np.ascontiguousarray(Wq[:, sl]),
                "wk": np.ascontiguousarray(Wk[:, sl]),
                "wv": np.ascontiguousarray(Wv[:, sl]),
                "wo": np.ascontiguousarray(Wo[sl, :]),
                "cosx": cosx,
                "sinx": sinx,
                "maskm": maskm,
            }
        )
    res = run_bass_kernel_spmd(_NC, in_maps, list(range(NCORES)), trace=TRACE)
    LAST_RESULT = res
    out = np.zeros((B, T, C), dtype=np.float32)
    for core in range(NCORES):
        b = core // (NCORES // B)
        out[b] += res.results[core]["out"]
    return out


# revision 4
# speedup vs baseline: 2.3062x; 2.3062x over previous
"""Trainium2 Bass kernel for windowed (inverted-window) attention.

Problem: B=2, T=2048, C=2048, H=16 heads, D=128, WINDOW=512.
  q,k,v = x@Wq, x@Wk, x@Wv  (per-head reshape), RoPE on q,k,
  scores masked so positions INSIDE the causal window are masked out
  (attend only to j>i or j<i-511), softmax, o@Wo.

Sharding: 8 cores = 2 (batch) x 4 (head groups of 4 heads).
Each core computes its batch's 4 heads end-to-end plus a partial
output projection (row-chunk of Wo); host sums the 4 partials per batch.
"""

import sys
import numpy as np

for _p in ("/opt/trn_rl_repo",):
    if _p not in sys.path:
        sys.path.insert(0, _p)

import concourse.bass as bass  # noqa: E402
import concourse.mybir as mybir  # noqa: E402
from concourse.bacc import Bacc  # noqa: E402
from concourse.tile import TileContext  # noqa: E402
from concourse.bass import ts, ds  # noqa: E402
from concourse.bass_utils import run_bass_kernel_spmd  # noqa: E402

B, T, C, H, D = 2, 2048, 2048, 16, 128
HL = 4                # heads per core
NCORES = 8
WINDOW = 512
ROPE_BASE = 10000.0
TB = 512              # i/t block size (matmul free dim)
NTB = T // TB         # 4
CK = C // 128         # 16 contraction chunks for projections
NTC = T // 128        # 16 j-chunks / t-chunks
MASK_OFF = 1920       # master strip offset: off = i0 - j0 + MASK_OFF
MASK_W = 4352
F32 = mybir.dt.float32
F32R = mybir.dt.float32r
AF = mybir.ActivationFunctionType

USE_F32R = True      # use full-rate float32r matmuls (vs 1/4-rate float32)
MM_DT = F32R if USE_F32R else F32   # dtype of every matmul operand tensor

_NC = None
TRACE = False
LAST_RESULT = None    # BassKernelResults of the most recent run (for test.py)




def build_nc():
    nc = Bacc()
    xT = nc.declare_dram_parameter("xT", [C, T], MM_DT, isOutput=False)
    wq = nc.declare_dram_parameter("wq", [C, HL * D], MM_DT, isOutput=False)
    wk = nc.declare_dram_parameter("wk", [C, HL * D], MM_DT, isOutput=False)
    wv = nc.declare_dram_parameter("wv", [C, HL * D], MM_DT, isOutput=False)
    wo = nc.declare_dram_parameter("wo", [HL * D, C], MM_DT, isOutput=False)
    cosx = nc.declare_dram_parameter("cosx", [128, T], F32, isOutput=False)
    sinx = nc.declare_dram_parameter("sinx", [128, T], F32, isOutput=False)
    maskm = nc.declare_dram_parameter("maskm", [128, MASK_W], F32, isOutput=False)
    out = nc.declare_dram_parameter("out", [T, C], F32, isOutput=True)

    xT_v = xT[:].rearrange("(co p) t -> p co t", p=128)   # [128, 16, T]
    wq_v = wq[:].rearrange("(co p) d -> p co d", p=128)   # [128, 16, 512]
    wk_v = wk[:].rearrange("(co p) d -> p co d", p=128)
    wv_v = wv[:].rearrange("(co p) d -> p co d", p=128)
    wo_v = wo[:].rearrange("(h p) c -> p h c", p=128)     # [128, 4, C]

    scale = float(1.0 / np.sqrt(D))

    with TileContext(nc) as tc:
        with tc.tile_pool(name="p0", bufs=1) as p0:
            oT = p0.tile([128, HL, T], MM_DT)       # per-head o, transposed [d, t]

            with tc.tile_pool(name="p1", bufs=1) as p1:
                V = p1.tile([128, NTC, HL * D], MM_DT)   # v natural [t, hd]
                QT = p1.tile([128, HL, T], MM_DT)        # q transposed [d, t]
                KT = p1.tile([128, HL, T], MM_DT)

                # ---- Phase A2: V = x @ Wv (natural layout) ----
                with (
                    tc.tile_pool(name="a2", bufs=1) as a2,
                    tc.tile_pool(name="a2x", bufs=2) as a2x,
                    tc.tile_pool(name="psV", bufs=2, space="PSUM") as psV,
                ):
                    wvt = a2.tile([128, CK, HL * D], MM_DT)
                    nc.sync.dma_start(wvt[:], wv_v[:])
                    for tch in range(NTC):
                        xc = a2x.tile([128, CK, 128], MM_DT, tag="xc")
                        nc.sync.dma_start(xc[:], xT_v[:, :, ts(tch, 128)])
                        ps = psV.tile([128, HL * D], F32, tag="psv")
                        for ck in range(CK):
                            nc.tensor.matmul(
                                ps[:], xc[:, ck, :], wvt[:, ck, :],
                                start=(ck == 0), stop=(ck == CK - 1),
                            )
                        nc.scalar.copy(V[:, tch, :], ps[:])

                # ---- Phase A1: QT/KT = (x @ Wq/Wk)^T with RoPE ----
                with (
                    tc.tile_pool(name="a1", bufs=1) as a1,
                    tc.tile_pool(name="a1w", bufs=2) as a1w,
                    tc.tile_pool(name="a1x", bufs=1) as a1x,
                    tc.tile_pool(name="a1t", bufs=2) as a1t,
                    tc.tile_pool(name="psA", bufs=4, space="PSUM") as psA,
                ):
                    cosb = a1.tile([128, T], F32)
                    nc.sync.dma_start(cosb[:], cosx[:])
                    sinb = a1.tile([128, T], F32)
                    nc.sync.dma_start(sinb[:], sinx[:])
                    for tb in range(NTB):
                        xtb = a1x.tile([128, CK, TB], MM_DT, tag="xtb")
                        nc.sync.dma_start(xtb[:], xT_v[:, :, ts(tb, TB)])
                        for h in range(HL):
                            for w_v, OUTT in ((wq_v, QT), (wk_v, KT)):
                                wt = a1w.tile([128, CK, D], MM_DT, tag="w")
                                nc.sync.dma_start(wt[:], w_v[:, :, ts(h, D)])
                                ps = psA.tile([128, TB], F32, tag="ps")
                                for ck in range(CK):
                                    nc.tensor.matmul(
                                        ps[:], wt[:, ck, :], xtb[:, ck, :],
                                        start=(ck == 0), stop=(ck == CK - 1),
                                    )
                                # RoPE: out = raw*cos + swap(raw)*sin_signed
                                raw = a1t.tile([128, TB], F32, tag="raw")
                                nc.scalar.copy(raw[:], ps[:])
                                sw = a1t.tile([128, TB], F32, tag="sw")
                                nc.sync.dma_start(sw[0:64, :], raw[64:128, :])
                                nc.sync.dma_start(sw[64:128, :], raw[0:64, :])
                                nc.vector.tensor_mul(sw[:], sw[:], sinb[:, ts(tb, TB)])
                                nc.vector.tensor_mul(raw[:], raw[:], cosb[:, ts(tb, TB)])
                                nc.vector.tensor_add(
                                    OUTT[:, h, ts(tb, TB)], sw[:], raw[:]
                                )

                # ---- Attention (per head, per i-block of 512 queries) ----
                with (
                    tc.tile_pool(name="att", bufs=1) as att,
                    tc.tile_pool(name="etp", bufs=18) as etp,
                    tc.tile_pool(name="smp", bufs=3) as smp,
                    tc.tile_pool(name="psS", bufs=3, space="PSUM") as psS,
                    tc.tile_pool(name="psO", bufs=2, space="PSUM") as psO,
                    tc.tile_pool(name="psZ", bufs=2, space="PSUM") as psZ,
                ):
                    maskb = att.tile([128, MASK_W], F32)
                    nc.sync.dma_start(maskb[:], maskm[:])
                    ones = att.tile([128, 128], MM_DT)
                    if USE_F32R:
                        ones_f = att.tile([128, 128], F32)
                        nc.vector.memset(ones_f[:], 1.0)
                        nc.vector.tensor_copy(ones[:], ones_f[:])
                    else:
                        nc.vector.memset(ones[:], 1.0)

                    for h in range(HL):
                        for ib in range(NTB):
                            ets = []
                            for c in range(NTC):
                                ps = psS.tile([128, TB], F32, tag="pss")
                                nc.tensor.matmul(
                                    ps[:],
                                    KT[:, h, ts(c, 128)],
                                    QT[:, h, ts(ib, TB)],
                                    start=True, stop=True,
                                )
                                et = etp.tile([128, TB], MM_DT, tag="et")
                                nc.scalar.activation(et[:], ps[:], AF.Exp, scale=scale)
                                dd = ib * TB - c * 128
                                if -(WINDOW - 1) <= dd <= (WINDOW - 1) + 127:
                                    off = dd + MASK_OFF
                                    nc.vector.tensor_mul(
                                        et[:], et[:], maskb[:, ds(off, TB)]
                                    )
                                ets.append(et)
                            pso = psO.tile([128, TB], F32, tag="pso")
                            psz = psZ.tile([128, TB], F32, tag="psz")
                            for c in range(NTC):
                                nc.tensor.matmul(
                                    pso[:], V[:, c, ts(h, D)], ets[c][:],
                                    start=(c == 0), stop=(c == NTC - 1),
                                )
                                nc.tensor.matmul(
                                    psz[:], ones[:], ets[c][:],
                                    start=(c == 0), stop=(c == NTC - 1),
                                )
                            rz = smp.tile([128, TB], F32, tag="rz")
                            nc.vector.reciprocal(rz[:], psz[:])
                            nc.vector.tensor_mul(oT[:, h, ts(ib, TB)], pso[:], rz[:])

            # ---- Phase C: partial output projection ----
            with (
                tc.tile_pool(name="oc", bufs=1) as oc,
                tc.tile_pool(name="ocb", bufs=3) as ocb,
                tc.tile_pool(name="psC", bufs=4, space="PSUM") as psC,
            ):
                wot = oc.tile([128, HL, C], MM_DT)
                nc.sync.dma_start(wot[:], wo_v[:])
                for tt in range(NTC):
                    for cb in range(NTB):
                        ps = psC.tile([128, TB], F32, tag="psc")
                        for h in range(HL):
                            nc.tensor.matmul(
                                ps[:],
                                oT[:, h, ts(tt, 128)],
                                wot[:, h, ts(cb, TB)],
                                start=(h == 0), stop=(h == HL - 1),
                            )
                        ob = ocb.tile([128, TB], F32, tag="ob")
                        nc.scalar.copy(ob[:], ps[:])
                        nc.sync.dma_start(out[ts(tt, 128), ts(cb, TB)], ob[:])

    nc.finalize()
    return nc


def _host_tables():
    inv_freq = (
        1.0 / (np.float32(ROPE_BASE) ** (np.arange(0, D, 2, dtype=np.float32) / np.float32(D)))
    ).astype(np.float32)
    t = np.arange(T, dtype=np.float32)
    freqs = (t[:, None] * inv_freq[None, :]).astype(np.float32)  # [T, 64]
    cos = np.cos(freqs).T.astype(np.float32)                     # [64, T]
    sin = np.sin(freqs).T.astype(np.float32)
    cosx = np.ascontiguousarray(np.concatenate([cos, cos], axis=0))      # [128, T]
    sinx = np.ascontiguousarray(np.concatenate([-sin, sin], axis=0))
    p = np.arange(128, dtype=np.int64)[:, None]
    u = np.arange(MASK_W, dtype=np.int64)[None, :]
    delta = u - MASK_OFF - p          # = i - j for tile offset
    allow = ~((delta >= 0) & (delta <= WINDOW - 1))
    maskm = np.ascontiguousarray(allow.astype(np.float32))
    return cosx, sinx, maskm


def kernel(x, Wq, Wk, Wv, Wo):
    global _NC, LAST_RESULT
    if _NC is None:
        _NC = build_nc()
    x = np.asarray(x, dtype=np.float32)
    Wq = np.asarray(Wq, dtype=np.float32)
    Wk = np.asarray(Wk, dtype=np.float32)
    Wv = np.asarray(Wv, dtype=np.float32)
    Wo = np.asarray(Wo, dtype=np.float32)
    cosx, sinx, maskm = _host_tables()
    in_maps = []
    for core in range(NCORES):
        b, hg = divmod(core, NCORES // B)
        sl = slice(hg * HL * D, (hg + 1) * HL * D)
        in_maps.append(
            {
                "xT": np.ascontiguousarray(x[b].T),
                "wq": np.ascontiguousarray(Wq[:, sl]),
                "wk": np.ascontiguousarray(Wk[:, sl]),
                "wv": np.ascontiguousarray(Wv[:, sl]),
                "wo": np.ascontiguousarray(Wo[sl, :]),
                "cosx": cosx,
                "sinx": sinx,
                "maskm": maskm,
            }
        )
    res = run_bass_kernel_spmd(_NC, in_maps, list(range(NCORES)), trace=TRACE)
    LAST_RESULT = res
    out = np.zeros((B, T, C), dtype=np.float32)
    for core in range(NCORES):
        b = core // (NCORES // B)
        out[b] += res.results[core]["out"]
    return out


# revision 7
# speedup vs baseline: 2.5160x; 1.0910x over previous
"""Trainium2 Bass kernel for windowed (inverted-window) attention.

Problem: B=2, T=2048, C=2048, H=16 heads, D=128, WINDOW=512.
  q,k,v = x@Wq, x@Wk, x@Wv  (per-head reshape), RoPE on q,k,
  scores masked so positions INSIDE the causal window are masked out
  (attend only to j>i or j<i-511), softmax, o@Wo.

Sharding: 8 cores = 2 (batch) x 4 (head groups of 4 heads).
Each core computes its batch's 4 heads end-to-end plus a partial
output projection (row-chunk of Wo); host sums the 4 partials per batch.
"""

import sys
import numpy as np

for _p in ("/opt/trn_rl_repo",):
    if _p not in sys.path:
        sys.path.insert(0, _p)

import concourse.bass as bass  # noqa: E402
import concourse.mybir as mybir  # noqa: E402
from concourse.bacc import Bacc  # noqa: E402
from concourse.tile import TileContext  # noqa: E402
from concourse.bass import ts, ds  # noqa: E402
from concourse.bass_utils import run_bass_kernel_spmd  # noqa: E402

B, T, C, H, D = 2, 2048, 2048, 16, 128
HL = 4                # heads per core
NCORES = 8
WINDOW = 512
ROPE_BASE = 10000.0
TB = 512              # i/t block size (matmul free dim)
NTB = T // TB         # 4
CK = C // 128         # 16 contraction chunks for projections
NTC = T // 128        # 16 j-chunks / t-chunks
MASK_OFF = 1920       # master strip offset: off = i0 - j0 + MASK_OFF
MASK_W = 4352
F32 = mybir.dt.float32
F32R = mybir.dt.float32r
AF = mybir.ActivationFunctionType

USE_F32R = True       # use full-rate float32r matmuls (vs 1/4-rate float32)
MM_DT = F32R if USE_F32R else F32   # dtype of every matmul operand tensor

_NC = None
TRACE = False
LAST_RESULT = None    # BassKernelResults of the most recent run (for test.py)


def _phase_a1(nc, tc, QT, KT, wq_v, wk_v, xT_v, cosx, sinx):
    """QT/KT = (x @ Wq/Wk)^T + RoPE, ck-outer with 8 PSUM accumulators."""
    with (
        tc.tile_pool(name="a1", bufs=1) as a1,
        tc.tile_pool(name="a1x", bufs=4) as a1x,
        tc.tile_pool(name="a1t", bufs=2) as a1t,
        tc.tile_pool(name="a1s", bufs=1) as a1s,
        tc.tile_pool(name="psA", bufs=1, space="PSUM") as psA,
    ):
        cosb = a1.tile([128, T], F32)
        nc.sync.dma_start(cosb[:], cosx[:])
        sinb = a1.tile([128, T], F32)
        nc.sync.dma_start(sinb[:], sinx[:])
        wqt = a1.tile([128, CK, HL * D], MM_DT)
        nc.sync.dma_start(wqt[:], wq_v[:])
        wkt = a1.tile([128, CK, HL * D], MM_DT)
        nc.sync.dma_start(wkt[:], wk_v[:])
        for tb in range(NTB):
            xbs = []
            for ck in range(CK):
                xb = a1x.tile([128, TB], MM_DT, tag="xtb", name=f"xb{tb}_{ck}")
                nc.gpsimd.dma_start(xb[:], xT_v[:, ck, ts(tb, TB)])
                xbs.append(xb)
            pss = []
            for i in range(2 * HL):
                pst = psA.tile([128, TB], F32, tag=f"ps{i}", name=f"pst{i}")
                pss.append(pst)
            for ck in range(CK):
                i = 0
                for h in range(HL):
                    for wt in (wqt, wkt):
                        nc.tensor.matmul(
                            pss[i][:], wt[:, ck, ts(h, D)], xbs[ck][:],
                            start=(ck == 0), stop=(ck == CK - 1),
                        )
                        i += 1
            i = 0
            for h in range(HL):
                for OUTT in (QT, KT):
                    ps = pss[i]
                    i += 1
                    # RoPE: out = raw*cos + swap(raw)*sin_signed
                    raw = a1t.tile([128, TB], F32, tag="raw")
                    nc.scalar.copy(raw[:], ps[:])
                    sw = a1s.tile([128, TB], F32, tag="sw")
                    nc.sync.dma_start(sw[0:64, :], raw[64:128, :])
                    nc.sync.dma_start(sw[64:128, :], raw[0:64, :])
                    nc.vector.tensor_mul(sw[:], sw[:], sinb[:, ts(tb, TB)])
                    nc.vector.tensor_mul(raw[:], raw[:], cosb[:, ts(tb, TB)])
                    nc.vector.tensor_add(OUTT[:, h, ts(tb, TB)], sw[:], raw[:])


def _phase_a2(nc, tc, V, wv_v, xT_v):
    """V = x @ Wv in natural [t, hd] layout."""
    with (
        tc.tile_pool(name="a2", bufs=1) as a2,
        tc.tile_pool(name="a2x", bufs=3) as a2x,
        tc.tile_pool(name="psV", bufs=2, space="PSUM") as psV,
    ):
        wvt = a2.tile([128, CK, HL * D], MM_DT)
        nc.sync.dma_start(wvt[:], wv_v[:])
        for tch in range(NTC):
            xc = a2x.tile([128, CK, 128], MM_DT, tag="xc")
            nc.sync.dma_start(xc[:], xT_v[:, :, ts(tch, 128)])
            ps = psV.tile([128, HL * D], F32, tag="psv")
            for ck in range(CK):
                nc.tensor.matmul(
                    ps[:], xc[:, ck, :], wvt[:, ck, :],
                    start=(ck == 0), stop=(ck == CK - 1),
                )
            nc.scalar.copy(V[:, tch, :], ps[:])


def _phase_att(nc, tc, oT, QT, KT, V, maskm, scale):
    """Transposed-scores attention with PE-side softmax denominator."""
    with (
        tc.tile_pool(name="att", bufs=1) as att,
        tc.tile_pool(name="etp", bufs=18) as etp,
        tc.tile_pool(name="smp", bufs=3) as smp,
        tc.tile_pool(name="psS", bufs=3, space="PSUM") as psS,
        tc.tile_pool(name="psO", bufs=2, space="PSUM") as psO,
        tc.tile_pool(name="psZ", bufs=2, space="PSUM") as psZ,
    ):
        maskb = att.tile([128, MASK_W], F32)
        nc.sync.dma_start(maskb[:], maskm[:])
        ones = att.tile([128, 128], MM_DT)
        if USE_F32R:
            ones_f = att.tile([128, 128], F32)
            nc.vector.memset(ones_f[:], 1.0)
            nc.vector.tensor_copy(ones[:], ones_f[:])
        else:
            nc.vector.memset(ones[:], 1.0)

        for h in range(HL):
            for ib in range(NTB):
                ets = []
                for c in range(NTC):
                    ps = psS.tile([128, TB], F32, tag="pss")
                    nc.tensor.matmul(
                        ps[:], KT[:, h, ts(c, 128)], QT[:, h, ts(ib, TB)],
                        start=True, stop=True,
                    )
                    et = etp.tile([128, TB], MM_DT, tag="et")
                    nc.scalar.activation(et[:], ps[:], AF.Exp, scale=scale)
                    dd = ib * TB - c * 128
                    if -(WINDOW - 1) <= dd <= (WINDOW - 1) + 127:
                        off = dd + MASK_OFF
                        nc.vector.tensor_mul(et[:], et[:], maskb[:, ds(off, TB)])
                    ets.append(et)
                pso = psO.tile([128, TB], F32, tag="pso")
                psz = psZ.tile([128, TB], F32, tag="psz")
                for c in range(NTC):
                    nc.tensor.matmul(
                        pso[:], V[:, c, ts(h, D)], ets[c][:],
                        start=(c == 0), stop=(c == NTC - 1),
                    )
                    nc.tensor.matmul(
                        psz[:], ones[:], ets[c][:],
                        start=(c == 0), stop=(c == NTC - 1),
                    )
                rz = smp.tile([128, TB], F32, tag="rz")
                nc.vector.reciprocal_approx_fast(rz[:], psz[:])
                nc.vector.tensor_mul(oT[:, h, ts(ib, TB)], pso[:], rz[:])


def _phase_oproj(nc, tc, oT, wo_v, out):
    """Partial output projection, cb-outer with streamed Wo column blocks."""
    with (
        tc.tile_pool(name="oc", bufs=2) as oc,
        tc.tile_pool(name="ocb", bufs=3) as ocb,
        tc.tile_pool(name="psC", bufs=4, space="PSUM") as psC,
    ):
        for cb in range(NTB):
            wot = oc.tile([128, HL, TB], MM_DT, tag="wot")
            nc.sync.dma_start(wot[:], wo_v[:, :, ts(cb, TB)])
            for tt in range(NTC):
                ps = psC.tile([128, TB], F32, tag="psc")
                for h in range(HL):
                    nc.tensor.matmul(
                        ps[:], oT[:, h, ts(tt, 128)], wot[:, h, :],
                        start=(h == 0), stop=(h == HL - 1),
                    )
                ob = ocb.tile([128, TB], F32, tag="ob")
                nc.scalar.copy(ob[:], ps[:])
                nc.sync.dma_start(out[ts(tt, 128), ts(cb, TB)], ob[:])


def build_nc():
    nc = Bacc()
    xT = nc.declare_dram_parameter("xT", [C, T], MM_DT, isOutput=False)
    wq = nc.declare_dram_parameter("wq", [C, HL * D], MM_DT, isOutput=False)
    wk = nc.declare_dram_parameter("wk", [C, HL * D], MM_DT, isOutput=False)
    wv = nc.declare_dram_parameter("wv", [C, HL * D], MM_DT, isOutput=False)
    wo = nc.declare_dram_parameter("wo", [HL * D, C], MM_DT, isOutput=False)
    cosx = nc.declare_dram_parameter("cosx", [128, T], F32, isOutput=False)
    sinx = nc.declare_dram_parameter("sinx", [128, T], F32, isOutput=False)
    maskm = nc.declare_dram_parameter("maskm", [128, MASK_W], F32, isOutput=False)
    out = nc.declare_dram_parameter("out", [T, C], F32, isOutput=True)

    xT_v = xT[:].rearrange("(co p) t -> p co t", p=128)   # [128, 16, T]
    wq_v = wq[:].rearrange("(co p) d -> p co d", p=128)   # [128, 16, 512]
    wk_v = wk[:].rearrange("(co p) d -> p co d", p=128)
    wv_v = wv[:].rearrange("(co p) d -> p co d", p=128)
    wo_v = wo[:].rearrange("(h p) c -> p h c", p=128)     # [128, 4, C]

    scale = float(1.0 / np.sqrt(D))

    with TileContext(nc) as tc:
        with tc.tile_pool(name="p0", bufs=1) as p0:
            oT = p0.tile([128, HL, T], MM_DT)   # per-head o, transposed [d, t]
            with tc.tile_pool(name="p1", bufs=1) as p1:
                QT = p1.tile([128, HL, T], MM_DT)    # q transposed [d, t]
                KT = p1.tile([128, HL, T], MM_DT)
                _phase_a1(nc, tc, QT, KT, wq_v, wk_v, xT_v, cosx, sinx)
                with tc.tile_pool(name="p2", bufs=1) as p2:
                    V = p2.tile([128, NTC, HL * D], MM_DT)   # v natural [t, hd]
                    _phase_a2(nc, tc, V, wv_v, xT_v)
                    _phase_att(nc, tc, oT, QT, KT, V, maskm, scale)
            _phase_oproj(nc, tc, oT, wo_v, out)

    nc.finalize()
    return nc


def _host_tables():
    inv_freq = (
        1.0 / (np.float32(ROPE_BASE) ** (np.arange(0, D, 2, dtype=np.float32) / np.float32(D)))
    ).astype(np.float32)
    t = np.arange(T, dtype=np.float32)
    freqs = (t[:, None] * inv_freq[None, :]).astype(np.float32)  # [T, 64]
    cos = np.cos(freqs).T.astype(np.float32)                     # [64, T]
    sin = np.sin(freqs).T.astype(np.float32)
    cosx = np.ascontiguousarray(np.concatenate([cos, cos], axis=0))      # [128, T]
    sinx = np.ascontiguousarray(np.concatenate([-sin, sin], axis=0))
    p = np.arange(128, dtype=np.int64)[:, None]
    u = np.arange(MASK_W, dtype=np.int64)[None, :]
    delta = u - MASK_OFF - p          # = i - j for tile offset
    allow = ~((delta >= 0) & (delta <= WINDOW - 1))
    maskm = np.ascontiguousarray(allow.astype(np.float32))
    return cosx, sinx, maskm


def kernel(x, Wq, Wk, Wv, Wo):
    global _NC, LAST_RESULT
    if _NC is None:
        _NC = build_nc()
    x = np.asarray(x, dtype=np.float32)
    Wq = np.asarray(Wq, dtype=np.float32)
    Wk = np.asarray(Wk, dtype=np.float32)
    Wv = np.asarray(Wv, dtype=np.float32)
    Wo = np.asarray(Wo, dtype=np.float32)
    cosx, sinx, maskm = _host_tables()
    in_maps = []
    for core in range(NCORES):
        b, hg = divmod(core, NCORES // B)
        sl = slice(hg * HL * D, (hg + 1) * HL * D)
        in_maps.append(
            {
                "xT": np.ascontiguousarray(x[b].T),
                "wq": np.ascontiguousarray(Wq[:, sl]),
                "wk": np.ascontiguousarray(Wk[:, sl]),
                "wv": np.ascontiguousarray(Wv[:, sl]),
                "wo": np.ascontiguousarray(Wo[sl, :]),
                "cosx": cosx,
                "sinx": sinx,
                "maskm": maskm,
            }
        )
    res = run_bass_kernel_spmd(_NC, in_maps, list(range(NCORES)), trace=TRACE)
    LAST_RESULT = res
    out = np.zeros((B, T, C), dtype=np.float32)
    for core in range(NCORES):
        b = core // (NCORES // B)
        out[b] += res.results[core]["out"]
    return out


# revision 8
# speedup vs baseline: 3.5350x; 1.4050x over previous
"""Trainium2 Bass kernel for windowed (inverted-window) attention.

Problem: B=2, T=2048, C=2048, H=16 heads, D=128, WINDOW=512.
  q,k,v = x@Wq, x@Wk, x@Wv  (per-head reshape), RoPE on q,k,
  scores masked so positions INSIDE the causal window are masked out
  (attend only to j>i or j<i-511), softmax, o@Wo.

Sharding: 8 cores = 2 (batch) x 4 (head groups of 4 heads).
Each core computes its batch's 4 heads end-to-end plus a partial
output projection (row-chunk of Wo); host sums the 4 partials per batch.

Matmul operands are bf16 (fp32 PSUM accumulation); everything else fp32.
"""

import sys
import numpy as np

for _p in ("/opt/trn_rl_repo",):
    if _p not in sys.path:
        sys.path.insert(0, _p)

import ml_dtypes  # noqa: E402
import concourse.bass as bass  # noqa: E402
import concourse.mybir as mybir  # noqa: E402
from concourse.bacc import Bacc  # noqa: E402
from concourse.tile import TileContext  # noqa: E402
from concourse.bass import ts, ds  # noqa: E402
from concourse.bass_utils import run_bass_kernel_spmd  # noqa: E402

B, T, C, H, D = 2, 2048, 2048, 16, 128
HL = 4                # heads per core
NCORES = 8
WINDOW = 512
ROPE_BASE = 10000.0
TB = 512              # i/t block size (matmul free dim)
NTB = T // TB         # 4
CK = C // 128         # 16 contraction chunks for projections
NTC = T // 128        # 16 j-chunks / t-chunks
MASK_OFF = 1920       # master strip offset: off = i0 - j0 + MASK_OFF
MASK_W = 4352
F32 = mybir.dt.float32
BF16 = mybir.dt.bfloat16
AF = mybir.ActivationFunctionType

MM_DT = BF16          # dtype of every matmul operand tensor
NP_MM = ml_dtypes.bfloat16

_NC = None
TRACE = False
LAST_RESULT = None    # BassKernelResults of the most recent run (for test.py)


def build_nc():
    nc = Bacc()
    xT = nc.declare_dram_parameter("xT", [C, T], MM_DT, isOutput=False)
    wq = nc.declare_dram_parameter("wq", [C, HL * D], MM_DT, isOutput=False)
    wk = nc.declare_dram_parameter("wk", [C, HL * D], MM_DT, isOutput=False)
    wv = nc.declare_dram_parameter("wv", [C, HL * D], MM_DT, isOutput=False)
    wo = nc.declare_dram_parameter("wo", [HL * D, C], MM_DT, isOutput=False)
    cosx = nc.declare_dram_parameter("cosx", [128, T], F32, isOutput=False)
    sinx = nc.declare_dram_parameter("sinx", [128, T], F32, isOutput=False)
    maskm = nc.declare_dram_parameter("maskm", [128, MASK_W], MM_DT, isOutput=False)
    out = nc.declare_dram_parameter("out", [T, C], F32, isOutput=True)

    xT_v = xT[:].rearrange("(co p) t -> p co t", p=128)   # [128, 16, T]
    wq_v = wq[:].rearrange("(co p) d -> p co d", p=128)   # [128, 16, 512]
    wk_v = wk[:].rearrange("(co p) d -> p co d", p=128)
    wv_v = wv[:].rearrange("(co p) d -> p co d", p=128)
    wo_v = wo[:].rearrange("(h p) c -> p h c", p=128)     # [128, 4, C]

    scale = float(1.0 / np.sqrt(D))

    with TileContext(nc) as tc:
        with (
            tc.tile_pool(name="res", bufs=1) as res,      # long-lived residents
            tc.tile_pool(name="xbp", bufs=20) as xbp,     # streamed x chunks
            tc.tile_pool(name="ropet", bufs=2) as ropet,
            tc.tile_pool(name="ropes", bufs=2) as ropes,
            tc.tile_pool(name="etp", bufs=18) as etp,
            tc.tile_pool(name="smp", bufs=3) as smp,
            tc.tile_pool(name="wop", bufs=2) as wop,
            tc.tile_pool(name="ocb", bufs=3) as ocb,
            tc.tile_pool(name="psum", bufs=1, space="PSUM") as psum,
        ):
            # ---- long-lived tensors; all big loads issued up-front ----
            wqt = res.tile([128, CK, HL * D], MM_DT)
            nc.sync.dma_start(wqt[:], wq_v[:])
            wkt = res.tile([128, CK, HL * D], MM_DT)
            nc.sync.dma_start(wkt[:], wk_v[:])
            wvt = res.tile([128, CK, HL * D], MM_DT)
            nc.sync.dma_start(wvt[:], wv_v[:])
            cosb = res.tile([128, T], F32)
            nc.sync.dma_start(cosb[:], cosx[:])
            sinb = res.tile([128, T], F32)
            nc.sync.dma_start(sinb[:], sinx[:])
            maskb = res.tile([128, MASK_W], MM_DT)
            nc.sync.dma_start(maskb[:], maskm[:])
            ones = res.tile([128, 128], MM_DT)
            nc.vector.memset(ones[:], 1.0)

            QT = res.tile([128, HL, T], MM_DT)    # q transposed [d, t]
            KT = res.tile([128, HL, T], MM_DT)
            V = res.tile([128, NTC, HL * D], MM_DT)   # v natural [t, hd]
            oT = res.tile([128, HL, T], MM_DT)    # per-head o transposed [d, t]

            # ---- Phase A: projections (QK transposed + RoPE, V natural) ----
            for tb in range(NTB):
                xbs = []
                for ck in range(CK):
                    xb = xbp.tile([128, TB], MM_DT, tag="xtb", name=f"xb{tb}_{ck}")
                    nc.gpsimd.dma_start(xb[:], xT_v[:, ck, ts(tb, TB)])
                    xbs.append(xb)
                pss = []
                for i in range(2 * HL):
                    pst = psum.tile([128, TB], F32, tag=f"pq{i}", name=f"pst{i}")
                    pss.append(pst)
                for ck in range(CK):
                    i = 0
                    for h in range(HL):
                        for wt in (wqt, wkt):
                            nc.tensor.matmul(
                                pss[i][:], wt[:, ck, ts(h, D)], xbs[ck][:],
                                start=(ck == 0), stop=(ck == CK - 1),
                            )
                            i += 1
                i = 0
                for h in range(HL):
                    for OUTT in (QT, KT):
                        ps = pss[i]
                        i += 1
                        # RoPE: out = raw*cos + swap(raw)*sin_signed
                        raw = ropet.tile([128, TB], F32, tag="raw")
                        nc.scalar.copy(raw[:], ps[:])
                        sw = ropes.tile([128, TB], F32, tag="sw")
                        nc.sync.dma_start(sw[0:64, :], raw[64:128, :])
                        nc.sync.dma_start(sw[64:128, :], raw[0:64, :])
                        nc.vector.tensor_mul(sw[:], sw[:], sinb[:, ts(tb, TB)])
                        nc.vector.tensor_mul(raw[:], raw[:], cosb[:, ts(tb, TB)])
                        nc.vector.tensor_add(OUTT[:, h, ts(tb, TB)], sw[:], raw[:])
                # V for the 4 t-chunks of this t-block (reuses the x chunks)
                for tco in range(NTB):
                    tch = tb * NTB + tco
                    psv = psum.tile(
                        [128, HL * D], F32, tag=f"pq{tco}", name=f"psv{tch}"
                    )
                    for ck in range(CK):
                        nc.tensor.matmul(
                            psv[:], xbs[ck][:, ts(tco, 128)], wvt[:, ck, :],
                            start=(ck == 0), stop=(ck == CK - 1),
                        )
                    nc.scalar.copy(V[:, tch, :], psv[:])

            # ---- Phase B: attention per (head, i-block) ----
            for h in range(HL):
                for ib in range(NTB):
                    ets = []
                    for c in range(NTC):
                        ps = psum.tile(
                            [128, TB], F32, tag=f"pq{c % 3}", name=f"pss{h}_{ib}_{c}"
                        )
                        nc.tensor.matmul(
                            ps[:], KT[:, h, ts(c, 128)], QT[:, h, ts(ib, TB)],
                            start=True, stop=True,
                        )
                        et = etp.tile([128, TB], MM_DT, tag="et")
                        nc.scalar.activation(et[:], ps[:], AF.Exp, scale=scale)
                        dd = ib * TB - c * 128
                        if -(WINDOW - 1) <= dd <= (WINDOW - 1) + 127:
                            off = dd + MASK_OFF
                            nc.vector.tensor_mul(et[:], et[:], maskb[:, ds(off, TB)])
                        ets.append(et)
                    pso = psum.tile([128, TB], F32, tag="pq4", name=f"pso{h}_{ib}")
                    psz = psum.tile([128, TB], F32, tag="pq5", name=f"psz{h}_{ib}")
                    for c in range(NTC):
                        nc.tensor.matmul(
                            pso[:], V[:, c, ts(h, D)], ets[c][:],
                            start=(c == 0), stop=(c == NTC - 1),
                        )
                        nc.tensor.matmul(
                            psz[:], ones[:], ets[c][:],
                            start=(c == 0), stop=(c == NTC - 1),
                        )
                    rz = smp.tile([128, TB], F32, tag="rz")
                    nc.vector.reciprocal_approx_fast(rz[:], psz[:])
                    nc.vector.tensor_mul(oT[:, h, ts(ib, TB)], pso[:], rz[:])

            # ---- Phase C: partial output projection (cb-outer) ----
            for cb in range(NTB):
                wot = wop.tile([128, HL, TB], MM_DT, tag="wot")
                nc.sync.dma_start(wot[:], wo_v[:, :, ts(cb, TB)])
                for tt in range(NTC):
                    ps = psum.tile(
                        [128, TB], F32, tag=f"pq{6 + tt % 2}", name=f"psc{cb}_{tt}"
                    )
                    for h in range(HL):
                        nc.tensor.matmul(
                            ps[:], oT[:, h, ts(tt, 128)], wot[:, h, :],
                            start=(h == 0), stop=(h == HL - 1),
                        )
                    ob = ocb.tile([128, TB], F32, tag="ob")
                    nc.scalar.copy(ob[:], ps[:])
                    nc.sync.dma_start(out[ts(tt, 128), ts(cb, TB)], ob[:])

    nc.finalize()
    return nc


def _host_tables():
    inv_freq = (
        1.0 / (np.float32(ROPE_BASE) ** (np.arange(0, D, 2, dtype=np.float32) / np.float32(D)))
    ).astype(np.float32)
    t = np.arange(T, dtype=np.float32)
    freqs = (t[:, None] * inv_freq[None, :]).astype(np.float32)  # [T, 64]
    cos = np.cos(freqs).T.astype(np.float32)                     # [64, T]
    sin = np.sin(freqs).T.astype(np.float32)
    cosx = np.ascontiguousarray(np.concatenate([cos, cos], axis=0))      # [128, T]
    sinx = np.ascontiguousarray(np.concatenate([-sin, sin], axis=0))
    p = np.arange(128, dtype=np.int64)[:, None]
    u = np.arange(MASK_W, dtype=np.int64)[None, :]
    delta = u - MASK_OFF - p          # = i - j for tile offset
    allow = ~((delta >= 0) & (delta <= WINDOW - 1))
    maskm = np.ascontiguousarray(allow.astype(NP_MM))
    return cosx, sinx, maskm


def kernel(x, Wq, Wk, Wv, Wo):
    global _NC, LAST_RESULT
    if _NC is None:
        _NC = build_nc()
    x = np.asarray(x, dtype=np.float32)
    Wq = np.asarray(Wq, dtype=np.float32)
    Wk = np.asarray(Wk, dtype=np.float32)
    Wv = np.asarray(Wv, dtype=np.float32)
    Wo = np.asarray(Wo, dtype=np.float32)
    cosx, sinx, maskm = _host_tables()
    in_maps = []
    for core in range(NCORES):
        b, hg = divmod(core, NCORES // B)
        sl = slice(hg * HL * D, (hg + 1) * HL * D)
        in_maps.append(
            {
                "xT": np.ascontiguousarray(x[b].T.astype(NP_MM)),
                "wq": np.ascontiguousarray(Wq[:, sl].astype(NP_MM)),
                "wk": np.ascontiguousarray(Wk[:, sl].astype(NP_MM)),
                "wv": np.ascontiguousarray(Wv[:, sl].astype(NP_MM)),
                "wo": np.ascontiguousarray(Wo[sl, :].astype(NP_MM)),
                "cosx": cosx,
                "sinx": sinx,
                "maskm": maskm,
            }
        )
    res = run_bass_kernel_spmd(_NC, in_maps, list(range(NCORES)), trace=TRACE)
    LAST_RESULT = res
    out = np.zeros((B, T, C), dtype=np.float32)
    for core in range(NCORES):
        b = core // (NCORES // B)
        out[b] += res.results[core]["out"]
    return out


# revision 11
# speedup vs baseline: 3.5939x; 1.0167x over previous
"""Trainium2 Bass kernel for windowed (inverted-window) attention.

Problem: B=2, T=2048, C=2048, H=16 heads, D=128, WINDOW=512.
  q,k,v = x@Wq, x@Wk, x@Wv  (per-head reshape), RoPE on q,k,
  scores masked so positions INSIDE the causal window are masked out
  (attend only to j>i or j<i-511), softmax, o@Wo.

Sharding: 8 cores = 2 (batch) x 4 (head groups of 4 heads).
Each core computes its batch's 4 heads end-to-end plus a partial
output projection (row-chunk of Wo); host sums the 4 partials per batch.

Matmul operands are bf16 (fp32 PSUM accumulation); everything else fp32.
"""

import sys
import numpy as np

for _p in ("/opt/trn_rl_repo",):
    if _p not in sys.path:
        sys.path.insert(0, _p)

import ml_dtypes  # noqa: E402
import concourse.bass as bass  # noqa: E402
import concourse.mybir as mybir  # noqa: E402
from concourse.bacc import Bacc  # noqa: E402
from concourse.tile import TileContext  # noqa: E402
from concourse.bass import ts, ds  # noqa: E402
from concourse.bass_utils import run_bass_kernel_spmd  # noqa: E402

B, T, C, H, D = 2, 2048, 2048, 16, 128
HL = 4                # heads per core
NCORES = 8
WINDOW = 512
ROPE_BASE = 10000.0
TB = 512              # i/t block size (matmul free dim)
NTB = T // TB         # 4
CK = C // 128         # 16 contraction chunks for projections
NTC = T // 128        # 16 j-chunks / t-chunks
MASK_OFF = 1920       # master strip offset: off = i0 - j0 + MASK_OFF
MASK_W = 4352
F32 = mybir.dt.float32
BF16 = mybir.dt.bfloat16
AF = mybir.ActivationFunctionType

MM_DT = BF16          # dtype of every matmul operand tensor
NP_MM = ml_dtypes.bfloat16

_NC = None
TRACE = False
LAST_RESULT = None    # BassKernelResults of the most recent run (for test.py)


def build_nc():
    nc = Bacc()
    xT = nc.declare_dram_parameter("xT", [C, T], MM_DT, isOutput=False)
    wq = nc.declare_dram_parameter("wq", [C, HL * D], MM_DT, isOutput=False)
    wk = nc.declare_dram_parameter("wk", [C, HL * D], MM_DT, isOutput=False)
    wv = nc.declare_dram_parameter("wv", [C, HL * D], MM_DT, isOutput=False)
    wo = nc.declare_dram_parameter("wo", [HL * D, C], MM_DT, isOutput=False)
    cosx = nc.declare_dram_parameter("cosx", [128, T], F32, isOutput=False)
    sinx = nc.declare_dram_parameter("sinx", [128, T], F32, isOutput=False)
    maskm = nc.declare_dram_parameter("maskm", [128, MASK_W], MM_DT, isOutput=False)
    out = nc.declare_dram_parameter("out", [T, C], F32, isOutput=True)

    xT_v = xT[:].rearrange("(co p) t -> p co t", p=128)   # [128, 16, T]
    wq_v = wq[:].rearrange("(co p) d -> p co d", p=128)   # [128, 16, 512]
    wk_v = wk[:].rearrange("(co p) d -> p co d", p=128)
    wv_v = wv[:].rearrange("(co p) d -> p co d", p=128)
    wo_v = wo[:].rearrange("(h p) c -> p h c", p=128)     # [128, 4, C]

    scale = float(1.0 / np.sqrt(D))

    with TileContext(nc) as tc:
        with (
            tc.tile_pool(name="res", bufs=1) as res,      # long-lived residents
            tc.tile_pool(name="xbp", bufs=20) as xbp,     # streamed x chunks
            tc.tile_pool(name="ropet", bufs=2) as ropet,
            tc.tile_pool(name="ropes", bufs=2) as ropes,
            tc.tile_pool(name="etp", bufs=17) as etp,
            tc.tile_pool(name="smp", bufs=3) as smp,
            tc.tile_pool(name="wop", bufs=2) as wop,
            tc.tile_pool(name="ocb", bufs=3) as ocb,
            tc.tile_pool(name="psum", bufs=1, space="PSUM") as psum,
        ):
            # ---- long-lived tensors; all big loads issued up-front ----
            wqt = res.tile([128, CK, HL * D], MM_DT)
            nc.sync.dma_start(wqt[:], wq_v[:])
            wkt = res.tile([128, CK, HL * D], MM_DT)
            nc.sync.dma_start(wkt[:], wk_v[:])
            wvt = res.tile([128, CK, HL * D], MM_DT)
            nc.sync.dma_start(wvt[:], wv_v[:])
            cosb = res.tile([128, T], F32)
            nc.sync.dma_start(cosb[:], cosx[:])
            sinb = res.tile([128, T], F32)
            nc.sync.dma_start(sinb[:], sinx[:])
            maskb = res.tile([128, MASK_W], MM_DT)
            nc.sync.dma_start(maskb[:], maskm[:])
            ones = res.tile([128, 128], MM_DT)
            nc.vector.memset(ones[:], 1.0)

            QT = res.tile([128, HL, T], MM_DT)    # q transposed [d, t]
            KT = res.tile([128, HL, T], MM_DT)
            V = res.tile([128, NTC, HL * D], MM_DT)   # v natural [t, hd]
            oT = res.tile([128, HL, T], MM_DT)    # per-head o transposed [d, t]

            # ---- Phase A: projections (QK transposed + RoPE, V natural) ----
            for tb in range(NTB):
                xbs = []
                for ck in range(CK):
                    xb = xbp.tile([128, TB], MM_DT, tag="xtb", name=f"xb{tb}_{ck}")
                    nc.gpsimd.dma_start(xb[:], xT_v[:, ck, ts(tb, TB)])
                    xbs.append(xb)
                pss = []
                for i in range(2 * HL):
                    pst = psum.tile([128, TB], F32, tag=f"pq{i}", name=f"pst{i}")
                    pss.append(pst)
                for ck in range(CK):
                    i = 0
                    for h in range(HL):
                        for wt in (wqt, wkt):
                            nc.tensor.matmul(
                                pss[i][:], wt[:, ck, ts(h, D)], xbs[ck][:],
                                start=(ck == 0), stop=(ck == CK - 1),
                            )
                            i += 1
                i = 0
                for h in range(HL):
                    for OUTT in (QT, KT):
                        ps = pss[i]
                        i += 1
                        # RoPE: out = raw*cos + swap(raw)*sin_signed
                        raw = ropet.tile([128, TB], F32, tag="raw")
                        nc.scalar.copy(raw[:], ps[:])
                        sw = ropes.tile([128, TB], F32, tag="sw")
                        nc.sync.dma_start(sw[0:64, :], raw[64:128, :])
                        nc.sync.dma_start(sw[64:128, :], raw[0:64, :])
                        nc.vector.tensor_mul(sw[:], sw[:], sinb[:, ts(tb, TB)])
                        nc.vector.tensor_mul(raw[:], raw[:], cosb[:, ts(tb, TB)])
                        nc.vector.tensor_add(OUTT[:, h, ts(tb, TB)], sw[:], raw[:])
                # V for the 4 t-chunks of this t-block (reuses the x chunks)
                for tco in range(NTB):
                    tch = tb * NTB + tco
                    psv = psum.tile(
                        [128, HL * D], F32, tag=f"pq{tco}", name=f"psv{tch}"
                    )
                    for ck in range(CK):
                        nc.tensor.matmul(
                            psv[:], xbs[ck][:, ts(tco, 128)], wvt[:, ck, :],
                            start=(ck == 0), stop=(ck == CK - 1),
                        )
                    nc.scalar.copy(V[:, tch, :], psv[:])

            # ---- Phase B: attention per (head, i-block) ----
            for h in range(HL):
                for ib in range(NTB):
                    ets = []
                    for c in range(NTC):
                        ps = psum.tile(
                            [128, TB], F32, tag=f"pq{c % 4}", name=f"pss{h}_{ib}_{c}"
                        )
                        nc.tensor.matmul(
                            ps[:], KT[:, h, ts(c, 128)], QT[:, h, ts(ib, TB)],
                            start=True, stop=True,
                        )
                        et = etp.tile([128, TB], MM_DT, tag="et")
                        nc.scalar.activation(et[:], ps[:], AF.Exp, scale=scale)
                        dd = ib * TB - c * 128
                        if -(WINDOW - 1) <= dd <= (WINDOW - 1) + 127:
                            off = dd + MASK_OFF
                            nc.vector.tensor_mul(et[:], et[:], maskb[:, ds(off, TB)])
                        ets.append(et)
                    pso = psum.tile([128, TB], F32, tag="pq4", name=f"pso{h}_{ib}")
                    psz = psum.tile([128, TB], F32, tag="pq5", name=f"psz{h}_{ib}")
                    for c in range(NTC):
                        nc.tensor.matmul(
                            pso[:], V[:, c, ts(h, D)], ets[c][:],
                            start=(c == 0), stop=(c == NTC - 1),
                        )
                        nc.tensor.matmul(
                            psz[:], ones[:], ets[c][:],
                            start=(c == 0), stop=(c == NTC - 1),
                        )
                    rz = smp.tile([128, TB], F32, tag="rz")
                    nc.vector.reciprocal_approx_fast(rz[:], psz[:])
                    nc.vector.tensor_mul(oT[:, h, ts(ib, TB)], pso[:], rz[:])

            # ---- Phase C: partial output projection (cb-outer) ----
            for cb in range(NTB):
                wot = wop.tile([128, HL, TB], MM_DT, tag="wot")
                nc.sync.dma_start(wot[:], wo_v[:, :, ts(cb, TB)])
                for tt in range(NTC):
                    ps = psum.tile(
                        [128, TB], F32, tag=f"pq{6 + tt % 2}", name=f"psc{cb}_{tt}"
                    )
                    for h in range(HL):
                        nc.tensor.matmul(
                            ps[:], oT[:, h, ts(tt, 128)], wot[:, h, :],
                            start=(h == 0), stop=(h == HL - 1),
                        )
                    ob = ocb.tile([128, TB], F32, tag="ob")
                    nc.scalar.copy(ob[:], ps[:])
                    nc.sync.dma_start(out[ts(tt, 128), ts(cb, TB)], ob[:])

    nc.finalize()
    return nc


def _host_tables():
    inv_freq = (
        1.0 / (np.float32(ROPE_BASE) ** (np.arange(0, D, 2, dtype=np.float32) / np.float32(D)))
    ).astype(np.float32)
    t = np.arange(T, dtype=np.float32)
    freqs = (t[:, None] * inv_freq[None, :]).astype(np.float32)  # [T, 64]
    cos = np.cos(freqs).T.astype(np.float32)                     # [64, T]
    sin = np.sin(freqs).T.astype(np.float32)
    cosx = np.ascontiguousarray(np.concatenate([cos, cos], axis=0))      # [128, T]
    sinx = np.ascontiguousarray(np.concatenate([-sin, sin], axis=0))
    p = np.arange(128, dtype=np.int64)[:, None]
    u = np.arange(MASK_W, dtype=np.int64)[None, :]
    delta = u - MASK_OFF - p          # = i - j for tile offset
    allow = ~((delta >= 0) & (delta <= WINDOW - 1))
    maskm = np.ascontiguousarray(allow.astype(NP_MM))
    return cosx, sinx, maskm


def kernel(x, Wq, Wk, Wv, Wo):
    global _NC, LAST_RESULT
    if _NC is None:
        _NC = build_nc()
    x = np.asarray(x, dtype=np.float32)
    Wq = np.asarray(Wq, dtype=np.float32)
    Wk = np.asarray(Wk, dtype=np.float32)
    Wv = np.asarray(Wv, dtype=np.float32)
    Wo = np.asarray(Wo, dtype=np.float32)
    cosx, sinx, maskm = _host_tables()
    in_maps = []
    for core in range(NCORES):
        b, hg = divmod(core, NCORES // B)
        sl = slice(hg * HL * D, (hg + 1) * HL * D)
        in_maps.append(
            {
                "xT": np.ascontiguousarray(x[b].T.astype(NP_MM)),
                "wq": np.ascontiguousarray(Wq[:, sl].astype(NP_MM)),
                "wk": np.ascontiguousarray(Wk[:, sl].astype(NP_MM)),
                "wv": np.ascontiguousarray(Wv[:, sl].astype(NP_MM)),
                "wo": np.ascontiguousarray(Wo[sl, :].astype(NP_MM)),
                "cosx": cosx,
                "sinx": sinx,
                "maskm": maskm,
            }
        )
    res = run_bass_kernel_spmd(_NC, in_maps, list(range(NCORES)), trace=TRACE)
    LAST_RESULT = res
    out = np.zeros((B, T, C), dtype=np.float32)
    for core in range(NCORES):
        b = core // (NCORES // B)
        out[b] += res.results[core]["out"]
    return out
